# revision 38
# baseline (speedup 1.0000x reference)
"""Additive (Bahdanau) attention on 8 Trainium2 cores.

Math: scores[b,q,k] = sum_e vT[e] * tanh(qp[b,q,e] + kp[b,k,e]);
out = softmax_k(scores) @ value.  qp = query @ Wq^T, kp = key @ Wk^T.

Direct evaluation needs B*Lq*Lk*E = 268M tanh's (ScalarE-bound, ~290us/core).
Instead approximate tanh(z) ~ c*z + sum_m b_m sin(m*alpha*z) on [-L, L]
(alpha = pi/L).  The angle-addition identity factorizes each harmonic:
    sin(m a (qp+kp)) = sin(m a qp) cos(m a kp) + cos(m a qp) sin(m a kp)
so scores becomes a sum of 2M rank-E products -> TensorE matmuls with
contraction dim E per harmonic (fp16 operands, fp32 PSUM accumulate).
Even harmonics come from double-angle products of odd ones.  The linear
term's q-part is constant per row (softmax-invariant, dropped); its
k-part is a per-k bias row injected via an extra one-hot fp32 chunk.
Measured output relative error vs the fp32 reference: ~1.4e-4.

Sharding: core = (batch, q-block): 2 batches x 4 q-blocks of 256 rows.
Each core computes its full attention independently; no collectives.
"""

import numpy as np

import concourse.bass as bass
import concourse.tile as tile
from concourse import mybir
from concourse.bass_utils import run_bass_kernel_spmd
from concourse.masks import make_identity

F32 = mybir.dt.float32
F32R = mybir.dt.float32r
F16 = mybir.dt.float16
AF = mybir.ActivationFunctionType
ALU = mybir.AluOpType

# ---- problem shapes (hardcoded per contract) ----
B, LQ, LK, D, E, VD = 2, 1024, 1024, 128, 128, 128
N_CORES = 8
QSH = (B * LQ) // N_CORES          # 256 q rows per core
NG = QSH // 128                    # 2 q-groups of 128 per core
NBLK = LK // 512                   # 2 k-blocks of 512

# ---- tanh Fourier approximation ----
L_PER = 7.0                        # half-period
M_HARM = 12                        # harmonics
ALPHA = float(np.pi / L_PER)
TWO_PI = float(np.float32(2 * np.pi))
HALF_PI = float(np.float32(np.pi / 2))
U_SCALE = float(1.0 / (2.0 * L_PER))   # angle in period units: u = proj/(2L)
MAGIC = 12582912.0                 # 1.5 * 2^23: x+MAGIC-MAGIC == round(x) in fp32


def _fourier_coeffs(L=L_PER, M=M_HARM, zfit=7.0, npts=20001):
    z = np.linspace(-zfit, zfit, npts)
    a = np.pi / L
    A = np.column_stack([z] + [np.sin(m * a * z) for m in range(1, M + 1)])
    coef, *_ = np.linalg.lstsq(A, np.tanh(z), rcond=None)
    return float(coef[0]), [float(b) for b in coef[1:]]


C_LIN, B_COEF = _fourier_coeffs()


def build_nc():
    nc = bass.Bass("TRN2", target_bir_lowering=False, debug=False)

    # Sin's float bias must be a pre-registered const AP (sundagen only
    # accepts immediate bias for Copy/Reciprocal).
    t = nc.alloc_sbuf_tensor("const-float32-halfpi", [128, 1], F32)
    nc.gpsimd.memset(t.ap(), HALF_PI)
    nc.const_aps.aps[(F32, HALF_PI)] = t.ap()
    nc.all_engine_barrier()

    q_d = nc.dram_tensor("q", [QSH, D], F32, kind="ExternalInput").ap()
    k_d = nc.dram_tensor("k", [LK, D], F32, kind="ExternalInput").ap()
    v_d = nc.dram_tensor("v", [LK, VD], F32, kind="ExternalInput").ap()
    w_d = nc.dram_tensor("w", [E, 2 * D], F32, kind="ExternalInput").ap()
    vt_d = nc.dram_tensor("vt", [E, 1], F32, kind="ExternalInput").ap()
    out_d = nc.dram_tensor("out", [QSH, VD], F32, kind="ExternalOutput").ap()

    with tile.TileContext(nc) as tc:
        _body(tc, q_d, k_d, v_d, w_d, vt_d, out_d)
    return nc


def _body(tc, q_d, k_d, v_d, w_d, vt_d, out_d):
    nc = tc.nc
    from contextlib import ExitStack
    ctx = ExitStack()
    with ctx:
        const = ctx.enter_context(tc.tile_pool(name="const", bufs=1))
        raw = ctx.enter_context(tc.tile_pool(name="raw", bufs=4))
        foldk = ctx.enter_context(tc.tile_pool(name="foldk", bufs=3))
        foldq = ctx.enter_context(tc.tile_pool(name="foldq", bufs=3))
        qraw = ctx.enter_context(tc.tile_pool(name="qraw", bufs=27))
        kfeat = ctx.enter_context(tc.tile_pool(name="kfeat", bufs=29))
        probs_p = ctx.enter_context(tc.tile_pool(name="probs", bufs=4))
        probsT_p = ctx.enter_context(tc.tile_pool(name="probsT", bufs=3))
        outp = ctx.enter_context(tc.tile_pool(name="outp", bufs=2))
        stat = ctx.enter_context(tc.tile_pool(name="stat", bufs=2))
        ps512 = ctx.enter_context(tc.tile_pool(name="ps512", bufs=4, space="PSUM"))
        ps128 = ctx.enter_context(tc.tile_pool(name="ps128", bufs=2, space="PSUM"))
        psav = ctx.enter_context(tc.tile_pool(name="psav", bufs=1, space="PSUM"))
        psrb = ctx.enter_context(tc.tile_pool(name="psrb", bufs=1, space="PSUM"))

        # ---------- constants ----------
        ident = const.tile([128, 128], F32, tag="ident")
        make_identity(nc, ident[:])

        w_sb = const.tile([E, 2 * D], F32, tag="w_sb")
        nc.sync.dma_start(w_sb[:], w_d[:])
        vt_sb = const.tile([E, 1], F32, tag="vt_sb")
        nc.sync.dma_start(vt_sb[:], vt_d[:])

        # value tiles [k-part, v] used directly as AV moving operand
        val = []
        for j in range(LK // 128):
            t = const.tile([128, VD], F32, tag=f"val{j}")
            nc.sync.dma_start(t[:], v_d[j * 128:(j + 1) * 128, :])
            val.append(t)

        # ---------- transposes: W halves, key, query ----------
        def transpose_to(dst_ap, src_ap):
            pt = ps128.tile([128, 128], F32, tag="tp")
            nc.tensor.transpose(pt[:], src_ap, ident[:])
            nc.vector.tensor_copy(dst_ap, pt[:])

        wqT = const.tile([D, E], F32, tag="wqT")
        transpose_to(wqT[:], w_sb[:, 0:D])
        wkT = const.tile([D, E], F32, tag="wkT")
        transpose_to(wkT[:], w_sb[:, D:2 * D])

        keyT = const.tile([D, LK], F32, tag="keyT")
        for j in range(LK // 128):
            kt = raw.tile([128, D], F32, tag="rawk")
            nc.sync.dma_start(kt[:], k_d[j * 128:(j + 1) * 128, :])
            transpose_to(keyT[:, j * 128:(j + 1) * 128], kt[:])

        queryT = const.tile([D, QSH], F32, tag="queryT")
        for j in range(QSH // 128):
            qt = raw.tile([128, D], F32, tag="rawq")
            nc.sync.dma_start(qt[:], q_d[j * 128:(j + 1) * 128, :])
            transpose_to(queryT[:, j * 128:(j + 1) * 128], qt[:])

        # ---------- projections -> base angle in PERIOD units ----------
        # u = proj/(2L), |u| <= ~0.34.  Harmonic m angle = frac(m*u) in
        # [-1/2, 1/2]; the ACT applies the 2*pi scale for free, keeping the
        # Sin spline input inside its valid range.
        base_k = const.tile([E, LK], F32, tag="base_k")
        for b in range(NBLK):
            pk = ps512.tile([128, 512], F32, tag="ps512")
            nc.tensor.matmul(pk[:], lhsT=wkT[:],
                             rhs=keyT[:, b * 512:(b + 1) * 512],
                             start=True, stop=True)
            nc.scalar.activation(base_k[:, b * 512:(b + 1) * 512], pk[:],
                                 AF.Copy, bias=0.0, scale=U_SCALE)

        base_q = const.tile([E, QSH], F32, tag="base_q")
        pq = ps512.tile([128, QSH], F32, tag="ps512")
        nc.tensor.matmul(pq[:], lhsT=wqT[:], rhs=queryT[:], start=True, stop=True)
        nc.scalar.activation(base_q[:], pq[:], AF.Copy, bias=0.0, scale=U_SCALE)

        # ---------- per-harmonic scale vectors ----------
        # Odd harmonics use raw sin/cos chunks scaled by b_m*vT.  Even
        # harmonics m=2j come from doubling: P_j = s_j*c_j = sin(2pi*m*u)/2
        # and Q_j = 2*s_j^2 = 1 - cos(2pi*m*u).  Expanding the products,
        # the q-only leftovers are softmax-invariant (dropped) and the
        # k-only leftover rows accumulate into the one-hot bias row.
        ODD = [m for m in range(1, M_HARM + 1, 2)]
        EVEN = [m for m in range(2, M_HARM + 1, 2)]
        ROUTE_B = {3, 7, 11}
        C1_DIRECT = True
        SQUARES_ON_POOL = False
        SQRT2 = float(np.sqrt(2.0))

        bv, bv2, bvn = {}, {}, {}
        for m in ODD:
            t = stat.tile([E, 1], F32, tag=f"bv{m}")
            nc.vector.tensor_scalar_mul(t[:], vt_sb[:], float(B_COEF[m - 1]))
            bv[m] = t
        for m in EVEN:
            t = stat.tile([E, 1], F32, tag=f"bv2{m}")
            nc.vector.tensor_scalar_mul(t[:], vt_sb[:], 2.0 * float(B_COEF[m - 1]))
            bv2[m] = t
            tn = stat.tile([E, 1], F32, tag=f"bvn{m}")
            nc.vector.tensor_scalar_mul(tn[:], vt_sb[:], -2.0 * float(B_COEF[m - 1]))
            bvn[m] = tn
        cvT = stat.tile([E, 1], F32, tag="cvT")
        nc.vector.tensor_scalar_mul(cvT[:], vt_sb[:], C_LIN * 2.0 * L_PER)

        def fold_step(prev_ap, step_ap, fold_pool, width):
            """frac(prev + step): add on Pool, round + sub on DVE."""
            t = fold_pool.tile([E, width], F32, tag="t")
            nc.gpsimd.tensor_add(t[:], prev_ap, step_ap)
            r = fold_pool.tile([E, width], F32, tag="r")
            nc.vector.tensor_scalar(r[:], t[:], MAGIC, MAGIC,
                                    op0=ALU.add, op1=ALU.subtract)
            wt = fold_pool.tile([E, width], F32, tag="w")
            nc.vector.tensor_sub(wt[:], t[:], r[:])
            return wt[:]

        def build_raws(u_ap, width, fold_pool, feat_pool, ftag):
            """Raw trig tiles: odd j -> (s_j, c_j); even source j -> P_j, Q_j.
            Chain w_{j+2} = frac(w_j + w_2); evens by doubling from j/2."""
            assert M_HARM == 12
            s, c, P, Q = {}, {}, {}, {}
            w = {1: u_ap}
            t2 = fold_pool.tile([E, width], F32, tag="t")
            nc.vector.tensor_scalar_mul(t2[:], u_ap, 2.0)
            r2 = fold_pool.tile([E, width], F32, tag="r")
            nc.vector.tensor_scalar(r2[:], t2[:], MAGIC, MAGIC,
                                    op0=ALU.add, op1=ALU.subtract)
            w2t = fold_pool.tile([E, width], F32, tag="w2")
            nc.vector.tensor_sub(w2t[:], t2[:], r2[:])
            w[2] = w2t[:]

            def odd_trig(j):
                sj = feat_pool.tile([E, width], F16, tag=ftag)
                nc.scalar.activation(sj[:], w[j], AF.Sin, scale=TWO_PI)
                cj = feat_pool.tile([E, width], F16, tag=ftag)
                if j == 1 and C1_DIRECT:
                    nc.scalar.activation(cj[:], w[j], AF.Sin, bias=HALF_PI,
                                         scale=TWO_PI)
                elif j in ROUTE_B:
                    ind = fold_pool.tile([E, width], F32, tag="tmp")
                    nc.vector.tensor_scalar(ind[:], w[j], 0.25, None, op0=ALU.is_ge)
                    v = fold_pool.tile([E, width], F32, tag="tmp")
                    nc.gpsimd.tensor_sub(v[:], w[j], ind[:])
                    nc.scalar.activation(cj[:], v[:], AF.Sin, bias=HALF_PI,
                                         scale=TWO_PI)
                else:
                    aw = fold_pool.tile([E, width], F32, tag="tmp")
                    nc.scalar.activation(aw[:], w[j], AF.Abs)
                    nc.scalar.activation(cj[:], aw[:], AF.Sin, bias=HALF_PI,
                                         scale=-TWO_PI)
                s[j], c[j] = sj, cj

            odd_trig(1)
            for j in range(3, M_HARM, 2):
                w[j] = fold_step(w[j - 2], w[2], fold_pool, width)
                odd_trig(j)

            def make_PQ(j, s_ap, c_ap):
                # P = s*c = sin(2pi*2j*u)/2 (fp16, Pool); Qa = 2 s^2 (fp32,
                # ACT); cE = 1 - Qa = cos(2pi*2j*u) exactly (fp16)
                p = feat_pool.tile([E, width], F16, tag=ftag)
                nc.gpsimd.tensor_mul(p[:], s_ap, c_ap)
                qa = fold_pool.tile([E, width], F32, tag="qa")
                nc.gpsimd.tensor_mul(qa[:], s_ap, s_ap)          # s^2 on Pool
                ce = feat_pool.tile([E, width], F16, tag=ftag)
                nc.vector.tensor_scalar(ce[:], qa[:], -2.0, 1.0,
                                        op0=ALU.mult, op1=ALU.add)
                P[j], Q[j] = p, qa
                C2[j] = ce

            C2 = {}
            make_PQ(1, s[1][:], c[1][:])
            for j in (2, 4):
                sr = fold_pool.tile([E, width], F32, tag="tmp")
                nc.vector.tensor_scalar_mul(sr[:], P[j // 2][:], 2.0)
                make_PQ(j, sr[:], C2[j // 2][:])
            make_PQ(3, s[3][:], c[3][:])
            sr6 = fold_pool.tile([E, width], F32, tag="tmp")
            nc.vector.tensor_scalar_mul(sr6[:], P[3][:], 2.0)
            make_PQ(6, sr6[:], C2[3][:])
            make_PQ(5, s[5][:], c[5][:])
            return s, c, P, C2

        # ---------- q-side features (stationary for scores matmuls) ----------
        sQ, cQ, Pq, Qq = build_raws(base_q[:], QSH, foldq, qraw, "feat")
        Fs, Fc = {}, {}
        for m in ODD:
            fs = const.tile([E, QSH], F16, tag=f"Fs{m}")
            nc.vector.tensor_scalar(fs[:], sQ[m][:], bv[m][:], None, op0=ALU.mult)
            fc = const.tile([E, QSH], F16, tag=f"Fc{m}")
            nc.vector.tensor_scalar(fc[:], cQ[m][:], bv[m][:], None, op0=ALU.mult)
            Fs[m], Fc[m] = fs, fc
        for m in EVEN:
            j = m // 2
            # F_A = 2 b vT * Pq  (= b vT sin_q);  F_B = 2 b vT (1 - Qq)/... =
            # bv2*(1 - Qq) = b vT * 2cos_q... see pairs below
            fa = const.tile([E, QSH], F16, tag=f"Fs{m}")
            nc.vector.tensor_scalar(fa[:], Pq[j][:], bv2[m][:], None, op0=ALU.mult)
            fb = const.tile([E, QSH], F16, tag=f"Fc{m}")
            nc.vector.tensor_scalar(fb[:], Qq[j][:], bv2[m][:], None, op0=ALU.mult)
            Fs[m], Fc[m] = fa, fb
        f_one = const.tile([E, QSH], F32, tag="f_one")
        nc.gpsimd.memset(f_one[:], 0.0)
        nc.gpsimd.memset(f_one[0:1, :], 1.0)

        # ---------- main: k-features per block + scores matmuls ----------
        score_ps = [[None] * NBLK for _ in range(NG)]
        for blk in range(NBLK):
            bk = base_k[:, blk * 512:(blk + 1) * 512]
            sK, cK, Pk, C2k = build_raws(bk, 512, foldk, kfeat, "G")

            # one-hot extra row: linear-term bias only (fp32 matmul)
            rbp = psrb.tile([1, 512], F32, tag="rb")
            nc.tensor.matmul(rbp[:], lhsT=cvT[:], rhs=bk, start=True, stop=True)
            gex = kfeat.tile([E, 512], F32, tag="Gex")
            nc.gpsimd.memset(gex[:], 0.0)
            nc.vector.tensor_copy(gex[0:1, :], rbp[:])

            pairs = []
            for m in ODD:
                pairs.append((Fs[m], cK[m]))     # b vT sin_q * cos_k
                pairs.append((Fc[m], sK[m]))     # b vT cos_q * sin_k
            for m in EVEN:
                j = m // 2
                pairs.append((Fs[m], C2k[j]))    # b vT sin_q * cos_k (exact)
                pairs.append((Fc[m], Pk[j]))     # 2 b vT cos_q * sin_k/2
            for g in range(NG):
                ps = ps512.tile([128, 512], F32, tag="ps512")
                score_ps[g][blk] = ps
                for ci, (f, gg) in enumerate(pairs):
                    nc.tensor.matmul(ps[:], lhsT=f[:, g * 128:(g + 1) * 128],
                                     rhs=gg[:], start=(ci == 0), stop=False)
                nc.tensor.matmul(ps[:], lhsT=f_one[:, g * 128:(g + 1) * 128],
                                 rhs=gex[:], start=False, stop=True)

        # ---------- softmax + AV per q-group ----------
        for g in range(NG):
            nmx = []
            for b in range(NBLK):
                t = stat.tile([128, 1], F32, tag="nmx")
                nc.vector.tensor_reduce(t[:], score_ps[g][b][:],
                                        axis=mybir.AxisListType.X,
                                        op=ALU.max, negate=True)
                nmx.append(t)
            nmg = stat.tile([128, 1], F32, tag="nmg")
            nc.vector.tensor_tensor(nmg[:], nmx[0][:], nmx[1][:], op=ALU.min)

            probs = []
            ssum = []
            for b in range(NBLK):
                p = probs_p.tile([128, 512], F32, tag="P")
                acc = stat.tile([128, 1], F32, tag="ssum")
                nc.scalar.activation(p[:], score_ps[g][b][:], AF.Exp, bias=nmg[:],
                                     accum_out=acc[:])
                probs.append(p)
                ssum.append(acc)
            stot = stat.tile([128, 1], F32, tag="stot")
            nc.vector.tensor_add(stot[:], ssum[0][:], ssum[1][:])
            rinv = stat.tile([128, 1], F32, tag="rinv")
            nc.vector.reciprocal(rinv[:], stot[:])

            pav = psav.tile([128, VD], F32, tag="av")
            for j in range(LK // 128):
                pt = ps128.tile([128, 128], F32, tag="tp")
                nc.tensor.transpose(pt[:], probs[j // 4][:, (j % 4) * 128:(j % 4 + 1) * 128],
                                    ident[:])
                pT = probsT_p.tile([128, 128], F32, tag="pT")
                nc.scalar.copy(pT[:], pt[:])
                nc.tensor.matmul(pav[:], lhsT=pT[:], rhs=val[j][:],
                                 start=(j == 0), stop=(j == LK // 128 - 1))

            osb = outp.tile([128, VD], F32, tag="osb")
            nc.vector.tensor_scalar(osb[:], pav[:], rinv[:], None, op0=ALU.mult)
            nc.sync.dma_start(out_d[g * 128:(g + 1) * 128, :], osb[:])


def _drop_trailing_range_clear(nc):
    """This walrus rejects the raw EVENT_SEMAPHORE_RANGE_CLEAR InstISA
    ("ISA wrong length").  Tile emits exactly one, at the kernel tail, to
    recycle pool semaphores for later tiles — of which there are none, so
    dropping it is safe.  Verified: no later instruction waits on the range."""
    import re
    for f in nc.m.functions:
        for blk in f.blocks:
            insts = list(blk.instructions)
            keep, pending = [], []
            for ins in insts:
                if (type(ins).__name__ == "InstISA"
                        and "EVENT_SEMAPHORE_RANGE_CLEAR" in ins.concise()):
                    m = re.search(r"range_first=(\d+) range_last=(\d+)", ins.concise())
                    pending.append((ins, set(range(int(m.group(1)), int(m.group(2)) + 1))))
                    continue
                for _, rng in pending:
                    si = ins.sync_info
                    if si is not None:
                        used = {w.id for w in si.on_wait} | {u.id for u in si.on_update}
                        assert not (used & rng), (
                            f"range-clear removal unsafe: {ins.name} uses {used & rng}")
                keep.append(ins)
            blk.instructions = keep


def split_excess_waits(nc, max_waits=1):
    """This walrus rejects >1 sync-wait per instruction; move extras onto
    preceding no-ops on the same engine (engines issue in order, so a wait
    on an earlier instruction subsumes one on the original)."""
    _drop_trailing_range_clear(nc)
    n = 0
    for f in nc.m.functions:
        for blk in f.blocks:
            new_list = []
            for ins in blk.instructions:
                si = ins.sync_info
                if si is not None and len(si.on_wait) > max_waits:
                    waits = list(si.on_wait)
                    extra, keep = waits[:-max_waits], waits[-max_waits:]
                    for j in range(0, len(extra), max_waits):
                        nop = mybir.InstNoOp(
                            name=f"{ins.name}-ws{j}",
                            engine=ins.engine,
                            sync_info=mybir.SyncInfo(on_wait=extra[j:j + max_waits],
                                                     on_update=[]),
                            bass_nofuse=True,
                        )
                        new_list.append(nop)
                    ins.sync_info = mybir.SyncInfo(on_wait=keep,
                                                  on_update=list(si.on_update))
                    n += 1
                new_list.append(ins)
            blk.instructions = new_list
    return n


_CACHED_NC = None


def _get_nc():
    global _CACHED_NC
    if _CACHED_NC is None:
        nc = build_nc()
        split_excess_waits(nc)
        _CACHED_NC = nc
    return _CACHED_NC


def make_in_maps(query, key, value, vT, weight):
    query = np.ascontiguousarray(np.asarray(query, np.float32))
    key = np.ascontiguousarray(np.asarray(key, np.float32))
    value = np.ascontiguousarray(np.asarray(value, np.float32))
    vT = np.ascontiguousarray(np.asarray(vT, np.float32)).reshape(E, 1)
    weight = np.ascontiguousarray(np.asarray(weight, np.float32))
    in_maps = []
    for c in range(N_CORES):
        b, qs = divmod(c, N_CORES // B)
        in_maps.append({
            "q": np.ascontiguousarray(query[b, qs * QSH:(qs + 1) * QSH]),
            "k": key[b],
            "v": value[b],
            "w": weight,
            "vt": vT,
        })
    return in_maps


def kernel(query, key, value, vT, weight):
    nc = _get_nc()
    in_maps = make_in_maps(query, key, value, vT, weight)
    res = run_bass_kernel_spmd(nc, in_maps, core_ids=list(range(N_CORES)))
    out = np.empty((B, LQ, VD), np.float32)
    for c in range(N_CORES):
        b, qs = divmod(c, N_CORES // B)
        out[b, qs * QSH:(qs + 1) * QSH] = res.results[c]["out"]
    return out
